# revision 1
# baseline (speedup 1.0000x reference)
"""Trainium2 Bass kernel for nn_Net_40561671143795.

Computation: xe = emb[x]; LSTM scan over T=512 (last hidden state);
out = h_T @ W_fc + b_fc.  B=4096, T=512, VOCAB=101, EMB=HID=32.

Sharding: batch split across 8 NeuronCores (512 rows each).

Per-core layout (all on-chip tensors):
  partition p = 32*u + q, u = batch-chunk (4 chunks of 128 cols), q = hid.
  free dim = batch columns within the chunk (W=128, split in NSTREAM streams).

Per step t:
  - GPSIMD ap_gather pulls xg = (emb@Wx + b)[x_t] for all 4 gate slots from a
    per-partition table (indices precomputed on host, wrapped 16-way).
  - identity matmul adds xg into PSUM; 4 block-diagonal Wh matmuls accumulate
    the recurrent term, giving gate pre-activations [128, 4 slots * w].
  - one Tanh(0.5*x) activation over all slots (sigmoid computed via tanh:
    sigma(x) = 0.5*tanh(x/2)+0.5; g-slot weights pre-doubled so it gets
    tanh(g) directly).
  - fused DVE ops: t1=(0.5*ti+0.5)*tg, t2=(0.5*tf+0.5)*c, c=t1+t2,
    tau_c=tanh(c) (ACT), h=(0.5*to+0.5)*tau_c.
"""

import numpy as np
import ml_dtypes

VOCAB, EMB, HID = 101, 32, 32
B, T = 4096, 512
NCORES = 8
B_LOC = B // NCORES          # 512
NCHUNK = 4                   # partition blocks of 32
W = B_LOC // NCHUNK          # 128 batch cols per chunk
NSTREAM = 2
WS = W // NSTREAM            # 64 cols per stream
NE = 4 * VOCAB               # 404 table entries per partition
IDX_W = NCHUNK * W // 16     # idx words per step per partition = 32

# slot order [i, f, o, g]; reference gate column bases in 4H: i=0, f=32, g=64, o=96
SLOT_BASE = [0, 32, 96, 64]
SLOT_MUL = [1.0, 1.0, 1.0, 2.0]  # g doubled for the sigma-via-tanh fold


def _host_prep(x, emb, Wx, Wh, b, W_fc):
    """Build device-side constant arrays + per-core index tables."""
    f32 = np.float32
    EW = (np.asarray(emb, f32) @ np.asarray(Wx, f32) + np.asarray(b, f32))  # [101, 128]
    Wh = np.asarray(Wh, f32)

    # gather table [128, 404]: table[32u+q, 101*g + v] = EW[v, base_g+q] * mul_g
    tab32 = np.empty((32, NE), f32)
    for g in range(4):
        tab32[:, 101 * g:101 * (g + 1)] = (
            EW[:, SLOT_BASE[g]:SLOT_BASE[g] + 32].T * SLOT_MUL[g]
        )
    table = np.tile(tab32, (4, 1))  # [128, 404]

    # block-diagonal Wh weights, fp16, slot order [i, f, o, g]
    bd = np.zeros((4, 128, 128), f32)
    for g in range(4):
        blk = Wh[:, SLOT_BASE[g]:SLOT_BASE[g] + 32] * SLOT_MUL[g]  # [32, 32]
        for u in range(NCHUNK):
            bd[g, 32 * u:32 * u + 32, 32 * u:32 * u + 32] = blk
    bd = bd.astype(np.float16)

    # FC head lhsT [128, 8]: wfc[32u+k, 2u+j] = W_fc[k, j]
    wfc = np.zeros((128, 8), f32)
    for u in range(NCHUNK):
        wfc[32 * u:32 * u + 32, 2 * u:2 * u + 2] = np.asarray(W_fc, f32)
    wfc = wfc.astype(np.float16)

    # per-core wrapped index tables [T, 128, IDX_W] int16
    # output col i (0..511) of the gather: s=i//(4*WS), g=(i%(4*WS))//WS, bcol=i%WS
    # batch-in-core = u*W + s*WS + bcol ; value = x[batch, t] + 101*g
    x = np.asarray(x)
    Tn = x.shape[1]
    i = np.arange(NCHUNK * W)             # 512 output cols
    s = i // (4 * WS)
    g = (i % (4 * WS)) // WS              # gate slot
    bcol = i % WS
    # per-partition-group u: batch index + slot offset
    # gather wrap: idxs[16c+pp, ss] -> col ss*16+pp for core c (u = c//2)
    pp = np.arange(16)
    ss = np.arange(IDX_W)
    col = ss[None, :] * 16 + pp[:, None]  # [16, IDX_W]
    idx_all = np.empty((NCORES, Tn, 128, IDX_W), np.int16)
    for core in range(NCORES):
        xc = x[core * B_LOC:(core + 1) * B_LOC]  # [512, T]
        for c16 in range(8):  # 8 gpsimd cores = partition groups of 16
            u = c16 // 2
            cc = col  # [16, IDX_W] output col ids
            batch = u * W + s[cc] * WS + bcol[cc]
            val = xc[batch] + 101 * g[cc][..., None]  # [16, IDX_W, T]
            idx_all[core, :, 16 * c16:16 * c16 + 16, :] = (
                val.transpose(2, 0, 1).astype(np.int16)
            )
    return table, bd, wfc, idx_all


def _build_program(Tn):
    """Build the Bass program (same for all cores)."""
    from contextlib import ExitStack
    import concourse.mybir as mybir
    from concourse import bacc
    from concourse.tile import TileContext

    f32 = mybir.dt.float32
    bf16 = mybir.dt.float16          # fp16 for tau/h/weights
    i16 = mybir.dt.int16
    AF = mybir.ActivationFunctionType

    nc = bacc.Bacc("TRN2", debug=False, enable_asserts=False)

    idx_d = nc.dram_tensor("idx", [Tn, 128, IDX_W], i16, kind="ExternalInput").ap()
    tab_d = nc.dram_tensor("tab", [128, NE], f32, kind="ExternalInput").ap()
    bd_d = nc.dram_tensor("bd", [4, 128, 128], bf16, kind="ExternalInput").ap()
    i128_d = nc.dram_tensor("i128", [128, 128], f32, kind="ExternalInput").ap()
    wfc_d = nc.dram_tensor("wfc", [128, 8], bf16, kind="ExternalInput").ap()
    out_d = nc.dram_tensor("out", [8, 128], f32, kind="ExternalOutput").ap()

    from concourse import library_config

    with TileContext(nc) as tc, ExitStack() as ctx:
        nc.gpsimd.load_library(library_config.ap_gather)
        const = ctx.enter_context(tc.tile_pool(name="const", bufs=1))
        state = ctx.enter_context(tc.tile_pool(name="state", bufs=1))
        work = ctx.enter_context(tc.tile_pool(name="work", bufs=3))
        psum = ctx.enter_context(tc.tile_pool(name="psum", bufs=2, space="PSUM"))
        psfc = ctx.enter_context(tc.tile_pool(name="psfc", bufs=1, space="PSUM"))

        # constants to SBUF
        tab_s = const.tile([128, NE], f32, name="tab_s")
        nc.sync.dma_start(tab_s, tab_d)
        idx_s = const.tile([128, Tn * IDX_W], i16, name="idx_s")
        nc.sync.dma_start(
            idx_s.rearrange("p (t w) -> p t w", t=Tn),
            idx_d.rearrange("t p w -> p t w"),
        )
        bd_s = [const.tile([128, 128], bf16, name=f"bd{g}_s") for g in range(4)]
        for g in range(4):
            nc.sync.dma_start(bd_s[g], bd_d[g])
        i128_s = const.tile([128, 128], f32, name="i128_s")
        nc.sync.dma_start(i128_s, i128_d)
        wfc_s = const.tile([128, 8], bf16, name="wfc_s")
        nc.sync.dma_start(wfc_s, wfc_d)

        # state
        h_s = state.tile([128, W], bf16, name="h_s")       # [s0 | s1]
        c_s = state.tile([128, W], f32, name="c_s")
        nc.vector.memset(h_s, 0.0)
        nc.vector.memset(c_s, 0.0)
        junk = state.tile([128, 1], f32, name="junk")

        # software pipeline: stream 1 runs half a step behind stream 0 so
        # that no in-order engine queue head ever waits on the other
        # stream's unfinished chain.
        def emit_front(s, t, xg):
            """MM (xg add + recurrent) then tanh of all gate slots.

            psum columns (per stream): WS-wide slots in order [i, f, o, 2g].
            Tile padded to a full 2KB PSUM bank (zero-region granularity)."""
            ps = psum.tile([128, 4 * WS], f32, name=f"ps{s}", tag=f"ps{s}",
                           padded_shape=[128, 512])
            nc.tensor.matmul(
                ps, i128_s, xg[:, 4 * WS * s:4 * WS * (s + 1)],
                start=True, stop=False,
            )
            for g in range(4):
                nc.tensor.matmul(
                    ps[:, WS * g:WS * (g + 1)], bd_s[g],
                    h_s[:, WS * s:WS * (s + 1)],
                    start=False, stop=(g == 3),
                )
            tau = work.tile([128, 4 * WS], bf16, name=f"tau{s}", tag=f"tau{s}")
            nc.scalar.activation(tau, ps, AF.Tanh, scale=0.5)
            return tau

        def emit_cell(s, tau):
            """c := sigma(f)*c + sigma(i)*tanh(g)  (DVE only)."""
            t1 = work.tile([128, WS], bf16, name=f"t1_{s}", tag=f"t1{s}")
            nc.vector.affine_mul_reduce(
                t1, junk, tau[:, 0:WS], tau[:, 3 * WS:4 * WS], 0.5, 0.5
            )
            t2 = work.tile([128, WS], f32, name=f"t2_{s}", tag=f"t2{s}")
            nc.vector.affine_mul_reduce(
                t2, junk, tau[:, WS:2 * WS], c_s[:, WS * s:WS * (s + 1)],
                0.5, 0.5,
            )
            nc.vector.tensor_tensor(
                c_s[:, WS * s:WS * (s + 1)], t1, t2, mybir.AluOpType.add
            )

        def emit_tail(s, tau):
            """tau_c then h := sigma(o)*tanh(c)."""
            tauc = work.tile([128, WS], bf16, name=f"tauc{s}", tag=f"tauc{s}")
            nc.scalar.activation(tauc, c_s[:, WS * s:WS * (s + 1)], AF.Tanh)
            nc.vector.affine_mul_reduce(
                h_s[:, WS * s:WS * (s + 1)], junk,
                tau[:, 2 * WS:3 * WS], tauc, 0.5, 0.5,
            )

        def gather(t):
            xg = work.tile([128, NCHUNK * W], f32, name="xg", tag="xg")
            nc.gpsimd.ap_gather(
                xg, tab_s, idx_s[:, t * IDX_W:(t + 1) * IDX_W],
                channels=128, num_elems=NE, d=1, num_idxs=NCHUNK * W,
            )
            return xg

        xg = gather(0)
        tau1_prev = None
        for t in range(Tn):
            tau0 = emit_front(0, t, xg)
            if tau1_prev is not None:
                emit_cell(1, tau1_prev)
                emit_tail(1, tau1_prev)
            emit_cell(0, tau0)
            tau1_prev = emit_front(1, t, xg)
            emit_tail(0, tau0)
            if t + 1 < Tn:
                xg = gather(t + 1)
        emit_cell(1, tau1_prev)
        emit_tail(1, tau1_prev)

        pfc = psfc.tile([8, W], f32, name="pfc")
        nc.tensor.matmul(pfc, wfc_s, h_s, start=True, stop=True)
        ofc = const.tile([8, W], f32, name="ofc")
        nc.vector.tensor_copy(ofc, pfc)
        nc.sync.dma_start(out_d, ofc)

    nc.compile()
    return nc


def _postprocess(outs, b_fc):
    """outs: list of 8 arrays [8, 128] -> [B, 2] f32."""
    res = np.empty((B, 2), np.float32)
    for core, o in enumerate(outs):
        for u in range(NCHUNK):
            blk = o[2 * u:2 * u + 2]  # [2, 128]
            rows = core * B_LOC + u * W
            res[rows:rows + W] = blk.T
    return res + np.asarray(b_fc, np.float32)


def kernel(x, emb, Wx, Wh, b, W_fc, b_fc):
    from concourse import bass_utils

    x = np.asarray(x)
    table, bd, wfc, idx_all = _host_prep(x, emb, Wx, Wh, b, W_fc)
    i128 = np.eye(128, dtype=np.float32)

    nc = _build_program(T)
    in_maps = [
        {
            "idx": np.ascontiguousarray(idx_all[core]),
            "tab": table,
            "bd": bd,
            "i128": i128,
            "wfc": wfc,
        }
        for core in range(NCORES)
    ]
    r = bass_utils.run_bass_kernel_spmd(nc, in_maps, core_ids=list(range(NCORES)))
    outs = [r.results[core]["out"] for core in range(NCORES)]
    return _postprocess(outs, b_fc)


if __name__ == "__main__":
    import reference

    inputs = {k: np.asarray(v) for k, v in reference.setup_inputs().items()}
    expected = np.asarray(reference.reference(**inputs))
    actual = kernel(**inputs)
    err = np.abs(actual - expected).max() / (np.abs(expected).max() + 1e-9)
    print("Relative error:", err)

